# revision 16
# baseline (speedup 1.0000x reference)
"""Trainium2 Bass kernel for nn_CombinedLoss (dice+CE+clDice+directional+conn+union).

Data-parallel over 8 NeuronCores: core c (b=c//4, q=c%4) owns D-planes
[16q,16q+16) of batch b and receives a replicate-padded 46-plane slab laid out
H-major [128 partitions, 46 planes, 128 W]. Replicate padding is equivalent to
the reference's SAME/-inf-pad window-shrink for 3-tap min/max pools. All
morphology runs in bf16 (exact on binary volumes) with shrinking per-iteration
plane ranges. Global sums accumulate per-partition via accum_out columns; the
per-batch rmax/rmin of skel_radius uses one 8-core AllReduce(max) of [1,8].
Host combines per-core partial sums into the final scalar.
"""
import os
import numpy as np

from concourse import bacc, bass_isa, mybir, tile
from concourse.bass_utils import run_bass_kernel_spmd

F32 = mybir.dt.float32
BF16 = mybir.dt.bfloat16
ALU = mybir.AluOpType
ACTF = mybir.ActivationFunctionType
AX = mybir.AxisListType

B, D, H, W = 2, 64, 128, 128
N_CORES = 8
HALO = 15
E = 16 + 2 * HALO      # 46 slab planes
CO = HALO              # core offset in slab
CW = 16                # core planes
EZ = CW + 2            # sobel slab planes (core +-1, zero padded)
NS = 18

(S_PROB, S_PROBY, S_Y, S_SOFTPLUS, S_YD, S_CONN0, S_CONN1, S_DIR,
 S_SKP, S_SKPY, S_SKT, S_SKTP,
 S_INTER1, S_QSP2, S_QSPQVL, S_INTER2, S_QSLQVP, S_QSL2) = range(NS)

_CACHED_NC = None


def _build_nc():
    nc = bacc.Bacc("TRN2", target_bir_lowering=False, debug=False,
                   num_devices=N_CORES)
    ins = {}
    for nm, shp in [("x0e", [128, E * W]), ("x1e", [128, E * W]),
                    ("tge", [128, E * W]), ("x0z", [128, EZ * W]),
                    ("yz", [128, EZ * W]), ("selv", [1, 8]),
                    ("negv", [1, 8]), ("sel01", [1, 8]),
                    ("band", [128, 128]), ("ident", [128, 128])]:
        ins[nm] = nc.dram_tensor(nm, shp, F32, kind="ExternalInput")
    sums_out = nc.dram_tensor("sums", [1, NS], F32, kind="ExternalOutput")
    with tile.TileContext(nc) as tc:
        _emit(nc, tc, ins, sums_out)
    nc.compile()
    return nc


def _emit(nc, tc, ins, sums_out):
    v, sc, gp = nc.vector, nc.scalar, nc.gpsimd
    A, Bc = CO, CO + CW

    with tc.tile_pool(name="persist", bufs=1) as pp, \
         tc.tile_pool(name="dram", bufs=1, space="DRAM") as dram:
        cols = pp.tile([128, NS], F32, tag="cols")

        def col(j):
            return cols[:, j:j + 1]

        # PE stationaries: band (3-tap H-sum w/ replicate-edge double-count),
        # identity, and negated band (for fused open-subtract)
        bandf = pp.tile([128, 128], F32, tag="bandf")
        identf = pp.tile([128, 128], F32, tag="identf")
        nc.sync.dma_start(out=bandf[:], in_=ins["band"][:])
        nc.sync.dma_start(out=identf[:], in_=ins["ident"][:])
        bandb = pp.tile([128, 128], BF16, tag="bandb")
        identb = pp.tile([128, 128], BF16, tag="identb")
        nbandb = pp.tile([128, 128], BF16, tag="nbandb")
        sc.copy(bandb[:], bandf[:])
        sc.copy(identb[:], identf[:])
        sc.mul(nbandb[:], bandf[:], -1.0)
        ebias = pp.tile([128, 1], F32, tag="ebias")
        obias = pp.tile([128, 1], F32, tag="obias")
        edbias = pp.tile([128, 1], F32, tag="edbias")
        v.memset(ebias[:], -1664.0)
        v.memset(obias[:], 128.0)
        v.memset(edbias[:], -6784.0)

        skp = pp.tile([128, CW, W], BF16, tag="skp")
        skt = pp.tile([128, CW, W], BF16, tag="skt")
        skh = pp.tile([128, CW, W], BF16, tag="skh")
        dit = pp.tile([128, CW, W], BF16, tag="dit")
        dip = pp.tile([128, CW, W], BF16, tag="dip")
        probb = pp.tile([128, CW, W], BF16, tag="probb")
        probc = pp.tile([128, CW, W], F32, tag="probc")
        yb = pp.tile([128, CW, W], BF16, tag="yb")
        bc = pp.tile([128, 8], F32, tag="bc")
        eps_col = pp.tile([128, 1], F32, tag="eps_col")
        v.memset(eps_col[:], 1e-4)

        with tc.tile_pool(name="ext", bufs=1) as pe:
            probe = pe.tile([128, E, W], BF16, tag="probe")
            harde = pe.tile([128, E, W], BF16, tag="harde")
            ye = pe.tile([128, E, W], BF16, tag="ye")

            # ------------- stage 0: loads, prob/hard/y, easy sums ----------
            with tc.tile_pool(name="s0", bufs=1) as p0:
                tgt = p0.tile([128, E, W], F32, tag="L3")
                nc.sync.dma_start(out=tgt[:], in_=ins["tge"][:].rearrange("p (a b) -> p a b", b=W))
                v.tensor_scalar(ye[:], tgt[:], 0.0, None, op0=ALU.is_gt)
                yc = p0.tile([128, CW, W], F32, tag="C1")
                v.tensor_scalar(yc[:], tgt[:, A:Bc, :], 0.0, 0.0,
                                op0=ALU.is_gt, op1=ALU.add,
                                accum_out=col(S_Y))
                sc.copy(yb[:], yc[:])

                x0t = p0.tile([128, E, W], F32, tag="L1")
                x1t = p0.tile([128, E, W], F32, tag="L2")
                nc.sync.dma_start(out=x0t[:], in_=ins["x0e"][:].rearrange("p (a b) -> p a b", b=W))
                nc.sync.dma_start(out=x1t[:], in_=ins["x1e"][:].rearrange("p (a b) -> p a b", b=W))
                scr = p0.tile([128, CW, W], F32, tag="C2")
                v.scalar_tensor_tensor(scr[:], x0t[:, A:Bc, :], 0.5, yc[:],
                                       op0=ALU.is_gt, op1=ALU.not_equal,
                                       accum_out=col(S_CONN0))
                v.scalar_tensor_tensor(scr[:], x1t[:, A:Bc, :], 0.5, yc[:],
                                       op0=ALU.is_gt, op1=ALU.not_equal,
                                       accum_out=col(S_CONN1))
                de = p0.tile([128, E, W], F32, tag="L3")  # reuses tgt slot
                v.tensor_tensor(de[:], x1t[:], x0t[:], op=ALU.subtract)
                v.scalar_tensor_tensor(scr[:], de[:, A:Bc, :], 1.0, yc[:],
                                       op0=ALU.mult, op1=ALU.mult,
                                       accum_out=col(S_YD))
                # softplus(d) = relu(d) + ln(1 + exp(-|d|))
                sp1 = p0.tile([128, CW, W], F32, tag="C3", name="sp1")
                sc.activation(sp1[:], de[:, A:Bc, :], ACTF.Abs)
                sp2 = p0.tile([128, CW, W], F32, tag="C2", name="sp2")
                sc.activation(sp2[:], sp1[:], ACTF.Exp, scale=-1.0)
                sp3 = p0.tile([128, CW, W], F32, tag="C3", name="sp3")
                sc.activation(sp3[:], sp2[:], ACTF.Ln, bias=1.0)
                sp4 = p0.tile([128, CW, W], F32, tag="C2", name="sp4")
                sc.activation(sp4[:], de[:, A:Bc, :], ACTF.Relu)
                sp5 = p0.tile([128, CW, W], F32, tag="C4", name="sp5")
                v.scalar_tensor_tensor(sp5[:], sp4[:], 1.0, sp3[:],
                                       op0=ALU.mult, op1=ALU.add,
                                       accum_out=col(S_SOFTPLUS))
                sc.activation(probe[:], de[:], ACTF.Sigmoid)
                v.tensor_scalar(harde[:], de[:], 0.0, None, op0=ALU.is_gt)
                sc.activation(probc[:], de[:, A:Bc, :], ACTF.Sigmoid,
                              accum_out=col(S_PROB))
                scr3 = p0.tile([128, CW, W], F32, tag="C3")
                v.scalar_tensor_tensor(scr3[:], probc[:], 1.0, yc[:],
                                       op0=ALU.mult, op1=ALU.mult,
                                       accum_out=col(S_PROBY))
                sc.copy(probb[:], probc[:])

            # ------------- morphology (bf16, shrinking ranges) -------------
            with tc.tile_pool(name="morph", bufs=1) as pm:
                m1 = pm.tile([128, E, W], BF16, tag="M1")
                m2 = pm.tile([128, E, W], BF16, tag="M2")
                dmh = pm.tile([128, E, W], BF16, tag="M4")
                sc_extra = pm.tile([128, E, W], BF16, tag="M8")
                ima = pm.tile([128, E, W], BF16, tag="M5")
                imb = pm.tile([128, E, W], BF16, tag="M6")
                # dedicated slabs/accumulators for the PE (tensor-engine)
                # skeleton pipeline so it runs concurrently with the DVE one
                pima = pm.tile([128, E, W], BF16, tag="P5")
                pimb = pm.tile([128, E, W], BF16, tag="P6")
                pca = pm.tile([128, CW, W], BF16, tag="pca")
                pcb = pm.tile([128, CW, W], BF16, tag="pcb")
                pd1 = pm.tile([128, CW, W], BF16, tag="pd1")
                opn = pm.tile([128, CW, W], BF16, tag="M7")
                ca = pm.tile([128, CW, W], BF16, tag="ca")
                cb = pm.tile([128, CW, W], BF16, tag="cb")
                d1 = pm.tile([128, CW, W], BF16, tag="d1")
                d2 = pm.tile([128, CW, W], BF16, tag="d2")

                def pool_w(op, dst, src, tmp, a, b):
                    v.tensor_tensor(tmp[:, a:b, 0:127], src[:, a:b, 0:127],
                                    src[:, a:b, 1:128], op=op)
                    sc.copy(tmp[:, a:b, 127:128], src[:, a:b, 127:128])
                    v.tensor_tensor(dst[:, a:b, 1:128], tmp[:, a:b, 0:127],
                                    tmp[:, a:b, 1:128], op=op)
                    sc.copy(dst[:, a:b, 0:1], tmp[:, a:b, 0:1])

                def pool_d(op, dst, src, tmp, a, b):
                    v.tensor_tensor(tmp[:, a:b, :], src[:, a - 1:b - 1, :],
                                    src[:, a:b, :], op=op)
                    v.tensor_tensor(dst[:, a:b, :], tmp[:, a:b, :],
                                    src[:, a + 1:b + 1, :], op=op)

                def pool_h(op, dst, src, dn, up, t1, a, b):
                    # both shifted copies issued up-front from src so the
                    # DMAs run in parallel and overlap compute
                    nc.sync.dma_start(out=dn[0:127, a:b, :],
                                      in_=src[1:128, a:b, :])
                    nc.sync.dma_start(out=dn[127:128, a:b, :],
                                      in_=src[127:128, a:b, :])
                    nc.sync.dma_start(out=up[1:128, a:b, :],
                                      in_=src[0:127, a:b, :])
                    nc.sync.dma_start(out=up[0:1, a:b, :],
                                      in_=src[0:1, a:b, :])
                    v.tensor_tensor(t1[:, a:b, :], src[:, a:b, :],
                                    dn[:, a:b, :], op=op)
                    v.tensor_tensor(dst[:, a:b, :], t1[:, a:b, :],
                                    up[:, a:b, :], op=op)

                def erode_cross(dst, src, a, b):
                    # sequential min-chain over the 7-point cross: H shifts
                    # via DMA copies (issued first, depend only on src), D/W
                    # via shifted APs; 6 big TTs total.
                    nc.sync.dma_start(out=dmh[0:127, a:b, :],
                                      in_=src[1:128, a:b, :])
                    nc.sync.dma_start(out=dmh[127:128, a:b, :],
                                      in_=src[127:128, a:b, :])
                    nc.sync.dma_start(out=sc_extra[1:128, a:b, :],
                                      in_=src[0:127, a:b, :])
                    nc.sync.dma_start(out=sc_extra[0:1, a:b, :],
                                      in_=src[0:1, a:b, :])
                    v.tensor_tensor(m1[:, a:b, :], src[:, a:b, :],
                                    dmh[:, a:b, :], op=ALU.min)
                    v.tensor_tensor(m2[:, a:b, :], m1[:, a:b, :],
                                    sc_extra[:, a:b, :], op=ALU.min)
                    v.tensor_tensor(m1[:, a:b, :], m2[:, a:b, :],
                                    src[:, a - 1:b - 1, :], op=ALU.min)
                    v.tensor_tensor(m2[:, a:b, :], m1[:, a:b, :],
                                    src[:, a + 1:b + 1, :], op=ALU.min)
                    v.tensor_tensor(m1[:, a:b, 1:128], m2[:, a:b, 1:128],
                                    src[:, a:b, 0:127], op=ALU.min)
                    sc.copy(m1[:, a:b, 0:1], m2[:, a:b, 0:1])
                    v.tensor_tensor(dst[:, a:b, 0:127], m1[:, a:b, 0:127],
                                    src[:, a:b, 1:128], op=ALU.min)
                    sc.copy(dst[:, a:b, 127:128], m1[:, a:b, 127:128])

                def box(op, dst, src, a, b):
                    pool_h(op, dmh, src, dmh, sc_extra, m2, a - 1, b + 1)
                    pool_w(op, m1, dmh, m2, a - 1, b + 1)
                    pool_d(op, dst, m1, m2, a, b)

                def box_core(op, dst_core, src):
                    # like box but writes only the CW core planes, indexed 0..
                    pool_h(op, dmh, src, dmh, sc_extra, m2, A - 1, Bc + 1)
                    pool_w(op, m1, dmh, m2, A - 1, Bc + 1)
                    v.tensor_tensor(m2[:, A:Bc, :], m1[:, A - 1:Bc - 1, :],
                                    m1[:, A:Bc, :], op=op)
                    v.tensor_tensor(dst_core[:], m2[:, A:Bc, :],
                                    m1[:, A + 1:Bc + 1, :], op=op)

                def skel_update(cold, cnew, img, first):
                    v.tensor_tensor(d1[:], img[:, A:Bc, :], opn[:],
                                    op=ALU.subtract)
                    sc.activation(d2[:], d1[:], ACTF.Relu)
                    v.tensor_scalar(d1[:], d2[:], -1.0, 1.0, op0=ALU.mult,
                                    op1=ALU.add)
                    if first:
                        sc.copy(cnew[:], d1[:])
                    else:
                        v.tensor_tensor(cnew[:], cold[:], d1[:], op=ALU.mult)

                MM = nc.tensor.matmul
                pctx = tc.tile_pool(name="psum", bufs=1, space="PSUM")
                pspool = pctx.__enter__()

                def erode_pe(dst, src, a, b):
                    # dst[:,a:b,:] = exact binary 7-point-cross erosion of src
                    # (reads src[a-1:b+1]); S = Hsum3 + D+-1 + W+-1 in PSUM,
                    # threshold S>=6.5 via saturating sigmoid (exact 0/1).
                    ci = 0
                    for c0 in range(a, b, 4):
                        c1 = min(c0 + 4, b)
                        n = c1 - c0
                        pt = pspool.tile([128, 4, W], F32, tag=f"ps{ci % 4}",
                                         name=f"er{c0}")
                        ci += 1
                        p = pt[:, 0:n, :]
                        MM(p, bandb[:], src[:, c0:c1, :],
                           start=True, stop=False)
                        MM(p, identb[:], src[:, c0 - 1:c1 - 1, :],
                           start=False, stop=False)
                        MM(p, identb[:], src[:, c0 + 1:c1 + 1, :],
                           start=False, stop=False)
                        MM(pt[:, 0:n, 0:127], identb[:], src[:, c0:c1, 1:128],
                           start=False, stop=False)
                        MM(pt[:, 0:n, 127:128], identb[:],
                           src[:, c0:c1, 127:128], start=False, stop=False)
                        MM(pt[:, 0:n, 1:128], identb[:], src[:, c0:c1, 0:127],
                           start=False, stop=False)
                        MM(pt[:, 0:n, 0:1], identb[:], src[:, c0:c1, 0:1],
                           start=False, stop=True)
                        sc.activation(dst[:, c0:c1, :], p, ACTF.Sigmoid,
                                      bias=ebias[:], scale=256.0)

                def open_update_pe(cold, cnew, img, ero, first):
                    # cnew = cold*(1 - relu(img - open(img))) over core planes;
                    # u = img - boxsum27(ero) in PSUM; pd1 = 1[u<0.5] exact.
                    ci = 0
                    for c0 in range(A, Bc, 4):
                        c1 = min(c0 + 4, Bc)
                        n = c1 - c0
                        pt = pspool.tile([128, 4, W], F32,
                                         tag=f"ps{ci % 4 + 4}", name=f"op{c0}")
                        p = pt[:, 0:n, :]
                        MM(p, identb[:], img[:, c0:c1, :],
                           start=True, stop=False)
                        for dd in (-1, 0, 1):
                            s0, s1 = c0 + dd, c1 + dd
                            MM(p, nbandb[:], ero[:, s0:s1, :],
                               start=False, stop=False)
                            MM(pt[:, 0:n, 0:127], nbandb[:],
                               ero[:, s0:s1, 1:128], start=False, stop=False)
                            MM(pt[:, 0:n, 127:128], nbandb[:],
                               ero[:, s0:s1, 127:128], start=False, stop=False)
                            MM(pt[:, 0:n, 1:128], nbandb[:],
                               ero[:, s0:s1, 0:127], start=False, stop=False)
                            MM(pt[:, 0:n, 0:1], nbandb[:], ero[:, s0:s1, 0:1],
                               start=False, stop=(dd == 1))
                        tgt = cnew if first else pd1
                        sc.activation(tgt[:, c0 - A:c1 - A, :], p,
                                      ACTF.Sigmoid, bias=obias[:],
                                      scale=-256.0)
                        ci += 1
                    if not first:
                        v.tensor_tensor(cnew[:], cold[:], pd1[:], op=ALU.mult)

                def pe_state(src):
                    return {"cur": src, "nxt": pimb, "ci": 0, "k": 0}

                def pe_step(st):
                    k = st["k"]
                    r = max(1, 11 - k)
                    cs = [pca, pcb]
                    erode_pe(st["nxt"], st["cur"], A - r, Bc + r)
                    open_update_pe(cs[1 - st["ci"]], cs[st["ci"]], st["cur"],
                                   st["nxt"], first=(k == 0))
                    st["ci"] = 1 - st["ci"]
                    st["cur"] = st["nxt"]
                    st["nxt"] = pima if st["cur"] is pimb else pimb
                    st["k"] = k + 1

                def pe_final(st, dst):
                    cs = [pca, pcb]
                    v.tensor_scalar(dst[:], cs[1 - st["ci"]][:], -1.0, 1.0,
                                    op0=ALU.mult, op1=ALU.add)

                def dve_state(src):
                    return {"cur": src, "nxt": imb, "ci": 0, "k": 0}

                def dve_step(st):
                    # one DVE skeleton iteration (erode + open + update)
                    k = st["k"]
                    r = max(1, 11 - k)
                    cs = [ca, cb]
                    erode_cross(st["nxt"], st["cur"], A - r, Bc + r)
                    box_core(ALU.max, opn, st["nxt"])
                    skel_update(cs[1 - st["ci"]], cs[st["ci"]], st["cur"],
                                first=(k == 0))
                    st["ci"] = 1 - st["ci"]
                    st["cur"] = st["nxt"]
                    st["nxt"] = ima if st["cur"] is imb else imb
                    st["k"] = k + 1

                def dve_final(st, dst):
                    cs = [ca, cb]
                    v.tensor_scalar(dst[:], cs[1 - st["ci"]][:], -1.0, 1.0,
                                    op0=ALU.mult, op1=ALU.add)

                def edt_state(src):
                    sc.copy(ca[:], src[:, A:Bc, :])
                    return {"cur": src, "nxt": imb, "ai": 0, "k": 1}

                def edt_step(st):
                    m = 15 - st["k"]
                    box(ALU.min, st["nxt"], st["cur"], A - m, Bc + m)
                    st["cur"] = st["nxt"]
                    st["nxt"] = ima if st["cur"] is imb else imb
                    accs = [ca, cb]
                    v.tensor_tensor(accs[1 - st["ai"]][:], accs[st["ai"]][:],
                                    st["cur"][:, A:Bc, :], op=ALU.add)
                    st["ai"] = 1 - st["ai"]
                    st["k"] += 1

                def edt_final(st, dst):
                    sc.copy(dst[:], [ca, cb][st["ai"]][:])

                def edt_pe_part(dst, src, a, b):
                    # dst[:,a:b,:] = exact binary 3x3x3 box-min of src via
                    # boxsum27 in PSUM + threshold >= 26.5
                    ci = 0
                    for c0 in range(a, b, 4):
                        c1 = min(c0 + 4, b)
                        n = c1 - c0
                        pt = pspool.tile([128, 4, W], F32, tag=f"ps{ci % 4}",
                                         name=f"eb{c0}")
                        ci += 1
                        p = pt[:, 0:n, :]
                        first = True
                        for dd in (-1, 0, 1):
                            s0, s1 = c0 + dd, c1 + dd
                            MM(p, bandb[:], src[:, s0:s1, :],
                               start=first, stop=False)
                            first = False
                            MM(pt[:, 0:n, 0:127], bandb[:],
                               src[:, s0:s1, 1:128], start=False, stop=False)
                            MM(pt[:, 0:n, 127:128], bandb[:],
                               src[:, s0:s1, 127:128],
                               start=False, stop=False)
                            MM(pt[:, 0:n, 1:128], bandb[:],
                               src[:, s0:s1, 0:127], start=False, stop=False)
                            MM(pt[:, 0:n, 0:1], bandb[:], src[:, s0:s1, 0:1],
                               start=False, stop=(dd == 1))
                        sc.activation(dst[:, c0:c1, :], p, ACTF.Sigmoid,
                                      bias=edbias[:], scale=256.0)

                def edt_hy_step(st):
                    # one box-min erosion split DVE (low planes) / PE (high).
                    # The acc-add for the PREVIOUS iteration is emitted after
                    # this one's DVE box so it never stalls the DVE queue
                    # waiting on the PE half (its inputs are long since done).
                    m = 15 - st["k"]
                    a, b = A - m, Bc + m
                    s = a + ((b - a) * 11 + 10) // 20
                    edt_pe_part(st["nxt"], st["cur"], s, b)
                    box(ALU.min, st["nxt"], st["cur"], a, s)
                    pend = st.get("pend")
                    if pend is not None:
                        accs = [ca, cb]
                        v.tensor_tensor(accs[1 - st["ai"]][:],
                                        accs[st["ai"]][:],
                                        pend[:, A:Bc, :], op=ALU.add)
                        st["ai"] = 1 - st["ai"]
                    st["cur"] = st["nxt"]
                    st["nxt"] = ima if st["cur"] is imb else imb
                    st["pend"] = st["cur"]
                    st["k"] += 1

                def edt_hy_flush(st):
                    accs = [ca, cb]
                    v.tensor_tensor(accs[1 - st["ai"]][:], accs[st["ai"]][:],
                                    st["pend"][:, A:Bc, :], op=ALU.add)
                    st["ai"] = 1 - st["ai"]
                    st["pend"] = None

                # phase 1: skt on PE/ACT concurrent with skp on DVE
                stT = pe_state(ye)
                stA = dve_state(probe)
                for k in range(11):
                    pe_step(stT)
                    dve_step(stA)
                pe_final(stT, skt)
                dve_final(stA, skp)

                # phase 2: skh on PE/ACT concurrent with edt(hard) on DVE
                stH = pe_state(harde)
                stE = edt_state(harde)
                for i in range(11):
                    pe_step(stH)
                    edt_step(stE)
                    if i < 4:
                        edt_step(stE)
                pe_final(stH, skh)
                edt_final(stE, d2)
                v.tensor_tensor(dip[:], d2[:], harde[:, A:Bc, :],
                                op=ALU.mult)

                # phase 3: edt(ye) split across DVE + PE each iteration
                stY = edt_state(ye)
                for _ in range(15):
                    edt_hy_step(stY)
                edt_hy_flush(stY)
                pctx.__exit__(None, None, None)
                edt_final(stY, d2)
                v.tensor_tensor(dit[:], d2[:], ye[:, A:Bc, :], op=ALU.mult)

                # cldice sums
                v.tensor_scalar(d1[:], skp[:], 1.0, 0.0, op0=ALU.mult,
                                op1=ALU.add, accum_out=col(S_SKP))
                v.scalar_tensor_tensor(d1[:], skp[:], 1.0, yb[:],
                                       op0=ALU.mult, op1=ALU.mult,
                                       accum_out=col(S_SKPY))
                v.tensor_scalar(d1[:], skt[:], 1.0, 0.0, op0=ALU.mult,
                                op1=ALU.add, accum_out=col(S_SKT))
                v.scalar_tensor_tensor(d1[:], skt[:], 1.0, probb[:],
                                       op0=ALU.mult, op1=ALU.mult,
                                       accum_out=col(S_SKTP))

        # ------------- stage 1.5: skel radii + AllReduce -------------------
        p2ctx = tc.tile_pool(name="s2", bufs=1)
        p2 = p2ctx.__enter__()
        sradt = p2.tile([128, CW, W], BF16, tag="sradt")
        spp = p2.tile([128, CW, W], F32, tag="spp")
        sind = p2.tile([128, CW, W], BF16, tag="sind")
        sradp = p2.tile([128, CW, W], BF16, tag="sradp")
        v.tensor_tensor(sradt[:], dit[:], skt[:], op=ALU.mult)
        skhf = p2.tile([128, CW, W], F32, tag="C0", name="skhf")
        sc.copy(skhf[:], skh[:])
        v.tensor_tensor(spp[:], skhf[:], probc[:], op=ALU.mult)
        v.tensor_scalar(sind[:], spp[:], 0.5, None, op0=ALU.is_gt)
        v.tensor_tensor(sradp[:], dip[:], sind[:], op=ALU.mult)

        mm = pp.tile([128, 4], F32, tag="mm")
        v.tensor_reduce(mm[:, 0:1], sradt[:], axis=AX.XY, op=ALU.max)
        v.tensor_reduce(mm[:, 1:2], sradp[:], axis=AX.XY, op=ALU.max)
        v.tensor_reduce(mm[:, 2:3], sradt[:], axis=AX.XY, op=ALU.min)
        v.tensor_reduce(mm[:, 3:4], sradp[:], axis=AX.XY, op=ALU.min)
        mm2 = pp.tile([128, 4], F32, tag="mm2")
        v.tensor_scalar(mm2[:, 0:2], mm[:, 0:2], 1.0, None, op0=ALU.mult)
        v.tensor_scalar(mm2[:, 2:4], mm[:, 2:4], -1.0, None, op0=ALU.mult)
        prm = pp.tile([128, 4], F32, tag="prm")
        gp.partition_all_reduce(prm[:], mm2[:], channels=128,
                                reduce_op=bass_isa.ReduceOp.max)
        my4 = prm[0:1, :]

        selt = pp.tile([1, 8], F32, tag="selt")
        negt = pp.tile([1, 8], F32, tag="negt")
        s01t = pp.tile([1, 8], F32, tag="s01t")
        nc.sync.dma_start(out=selt[:], in_=ins["selv"][:])
        nc.sync.dma_start(out=negt[:], in_=ins["negv"][:])
        nc.sync.dma_start(out=s01t[:], in_=ins["sel01"][:])
        tile8 = pp.tile([1, 8], F32, tag="tile8")
        sc.copy(tile8[:, 0:4], my4)
        sc.copy(tile8[:, 4:8], my4)
        arin = pp.tile([1, 8], F32, tag="arin")
        v.tensor_tensor(arin[:], tile8[:], selt[:], op=ALU.mult)
        v.tensor_tensor(tile8[:], arin[:], negt[:], op=ALU.add)

        ccin = dram.tile([1, 8], F32)
        ccout = dram.tile([1, 8], F32, addr_space="Shared")
        nc.sync.dma_start(out=ccin[:], in_=tile8[:])
        if os.environ.get("KERNEL_NO_CC"):
            nc.sync.dma_start(out=ccout[:], in_=ccin[:])
        else:
            gp.collective_compute("AllReduce", ALU.max,
                                  replica_groups=[list(range(N_CORES))],
                                  ins=[ccin[:]], outs=[ccout[:]])
        rv = pp.tile([1, 8], F32, tag="rv")
        nc.sync.dma_start(out=rv[:], in_=ccout[:])

        rvm = pp.tile([1, 8], F32, tag="rvm")
        v.tensor_tensor(rvm[:], rv[:], s01t[:], op=ALU.mult)
        my4r = pp.tile([1, 4], F32, tag="my4r")
        v.tensor_reduce(my4r[:], rvm[:].rearrange("p (a b) -> p b a", a=2),
                        axis=AX.X, op=ALU.add)
        rmx = pp.tile([1, 4], F32, tag="rmx")
        v.tensor_scalar(rmx[:, 0:2], my4r[:, 0:2], 1.0, None, op0=ALU.max)
        v.tensor_scalar(rmx[:, 2:4], my4r[:, 2:4], -1.0, 1.0, op0=ALU.mult,
                        op1=ALU.max)
        inv = pp.tile([1, 4], F32, tag="inv")
        v.reciprocal(inv[:, 0:2], rmx[:, 0:2])
        # bc8: [rmax_t, inv_t, -inv_t, 1+rmin_t*inv_t,
        #       rmax_p, inv_p, -inv_p, 1+rmin_p*inv_p]
        bc8 = pp.tile([1, 8], F32, tag="bc8")
        sc.copy(bc8[:, 0:1], rmx[:, 0:1])
        sc.copy(bc8[:, 1:2], inv[:, 0:1])
        sc.activation(bc8[:, 2:3], inv[:, 0:1], ACTF.Copy, scale=-1.0)
        t11 = pp.tile([1, 2], F32, tag="t11")
        v.scalar_tensor_tensor(t11[:, 0:1], rmx[:, 2:3], 1.0, inv[:, 0:1],
                               op0=ALU.mult, op1=ALU.mult)
        v.tensor_scalar(bc8[:, 3:4], t11[:, 0:1], 1.0, None, op0=ALU.add)
        sc.copy(bc8[:, 4:5], rmx[:, 1:2])
        sc.copy(bc8[:, 5:6], inv[:, 1:2])
        sc.activation(bc8[:, 6:7], inv[:, 1:2], ACTF.Copy, scale=-1.0)
        v.scalar_tensor_tensor(t11[:, 1:2], rmx[:, 3:4], 1.0, inv[:, 1:2],
                               op0=ALU.mult, op1=ALU.mult)
        v.tensor_scalar(bc8[:, 7:8], t11[:, 1:2], 1.0, None, op0=ALU.add)
        gp.partition_broadcast(bc[:], bc8[:])

        # ------------- stage 2: union-loss sums ----------------------------
        if True:
            C = [p2.tile([128, CW, W], F32, tag=f"C{i}", name=f"C{i}")
                 for i in range(7)]
            # pair 1: q_sp (pred path) with q_vl (true path)
            sc.copy(C[0][:], dit[:])
            qvl = C[1]
            v.tensor_scalar(qvl[:], C[0][:], bc[:, 0:1], bc[:, 1:2],
                            op0=ALU.min, op1=ALU.mult)
            sc.copy(C[0][:], sradp[:])
            v.tensor_scalar(C[2][:], C[0][:], bc[:, 6:7], bc[:, 7:8],
                            op0=ALU.mult, op1=ALU.add)    # u_p
            sc.square(C[3][:], C[2][:])
            sc.copy(C[0][:], sind[:])
            v.tensor_tensor(C[2][:], C[3][:], C[0][:], op=ALU.mult)
            qsp = C[3]
            v.tensor_tensor(qsp[:], C[2][:], spp[:], op=ALU.mult)
            sc.activation(C[0][:], qsp[:], ACTF.Ln, bias=eps_col[:])
            sc.activation(C[2][:], C[0][:], ACTF.Exp, scale=0.7)  # (qsp+eps)^.7
            v.tensor_tensor(C[4][:], qsp[:], qvl[:], op=ALU.mult)
            v.scalar_tensor_tensor(C[5][:], C[4][:], 1.0, C[2][:],
                                   op0=ALU.mult, op1=ALU.mult,
                                   accum_out=col(S_INTER1))
            sc.activation(C[5][:], qsp[:], ACTF.Square, accum_out=col(S_QSP2))
            v.tensor_scalar(C[5][:], C[4][:], 1.0, 0.0, op0=ALU.mult,
                            op1=ALU.add, accum_out=col(S_QSPQVL))
            # pair 2: q_sl (true path) with q_vp (pred path)
            sc.copy(C[0][:], sradt[:])
            v.tensor_scalar(C[2][:], C[0][:], bc[:, 2:3], bc[:, 3:4],
                            op0=ALU.mult, op1=ALU.add)    # u_t
            sc.square(C[4][:], C[2][:])
            sc.copy(C[0][:], skt[:])
            qsl = C[2]
            v.tensor_tensor(qsl[:], C[4][:], C[0][:], op=ALU.mult)
            sc.copy(C[0][:], dip[:])
            v.tensor_scalar(C[4][:], C[0][:], bc[:, 4:5], bc[:, 5:6],
                            op0=ALU.min, op1=ALU.mult)
            qvp = C[5]
            v.tensor_tensor(qvp[:], C[4][:], probc[:], op=ALU.mult)
            sc.activation(C[0][:], qvp[:], ACTF.Ln, bias=eps_col[:])
            sc.activation(C[4][:], C[0][:], ACTF.Exp, scale=0.7)  # (qvp+eps)^.7
            sc.activation(C[0][:], qsl[:], ACTF.Square, accum_out=col(S_QSL2))
            v.scalar_tensor_tensor(C[6][:], C[0][:], 1.0, C[4][:],
                                   op0=ALU.mult, op1=ALU.mult,
                                   accum_out=col(S_INTER2))
            v.scalar_tensor_tensor(C[6][:], qsl[:], 1.0, qvp[:],
                                   op0=ALU.mult, op1=ALU.mult,
                                   accum_out=col(S_QSLQVP))

        p2ctx.__exit__(None, None, None)

        # ------------- sobel / directional ---------------------------------
        with tc.tile_pool(name="sob", bufs=1) as ps:
            T = [ps.tile([128, EZ, W], F32, tag=f"Z{i}", name=f"Z{i}")
                 for i in range(12)]
            (x0zt, yzt, sA, sB, sC, dmt, gx, gy, gz, tx, ty, tz) = T
            nc.sync.dma_start(out=x0zt[:], in_=ins["x0z"][:].rearrange("p (a b) -> p a b", b=W))
            nc.sync.dma_start(out=yzt[:], in_=ins["yz"][:].rearrange("p (a b) -> p a b", b=W))
            c0, c1 = 1, EZ - 1

            def d1_w(dst, src, a, b):
                v.tensor_tensor(dst[:, a:b, 1:127], src[:, a:b, 2:128],
                                src[:, a:b, 0:126], op=ALU.subtract)
                sc.copy(dst[:, a:b, 0:1], src[:, a:b, 1:2])
                sc.activation(dst[:, a:b, 127:128], src[:, a:b, 126:127],
                              ACTF.Copy, scale=-1.0)

            def s1_d(dst, src, tmp, a, b):
                v.tensor_tensor(tmp[:, a:b, :], src[:, a - 1:b - 1, :],
                                src[:, a:b, :], op=ALU.add)
                v.tensor_tensor(dst[:, a:b, :], tmp[:, a:b, :],
                                src[:, a + 1:b + 1, :], op=ALU.add)

            def s2_d(dst, src, tmp, a, b):
                v.tensor_tensor(tmp[:, a:b, :], src[:, a - 1:b - 1, :],
                                src[:, a + 1:b + 1, :], op=ALU.add)
                v.scalar_tensor_tensor(dst[:, a:b, :], src[:, a:b, :], 2.0,
                                       tmp[:, a:b, :], op0=ALU.mult,
                                       op1=ALU.add)

            def s2_w(dst, src, tmp, a, b):
                v.tensor_tensor(tmp[:, a:b, 1:127], src[:, a:b, 0:126],
                                src[:, a:b, 2:128], op=ALU.add)
                v.scalar_tensor_tensor(dst[:, a:b, 1:127],
                                       src[:, a:b, 1:127], 2.0,
                                       tmp[:, a:b, 1:127], op0=ALU.mult,
                                       op1=ALU.add)
                v.scalar_tensor_tensor(dst[:, a:b, 0:1], src[:, a:b, 0:1],
                                       2.0, src[:, a:b, 1:2], op0=ALU.mult,
                                       op1=ALU.add)
                v.scalar_tensor_tensor(dst[:, a:b, 127:128],
                                       src[:, a:b, 127:128], 2.0,
                                       src[:, a:b, 126:127], op0=ALU.mult,
                                       op1=ALU.add)

            def shift_h(dst, src, a, b, up):
                sc.memzero(dst[:, a:b, :])
                if up:
                    nc.sync.dma_start(out=dst[1:128, a:b, :],
                                      in_=src[0:127, a:b, :])
                else:
                    nc.sync.dma_start(out=dst[0:127, a:b, :],
                                      in_=src[1:128, a:b, :])

            def s_h(dst, src, tdma, t2, a, b, two_center):
                shift_h(tdma, src, a, b, up=False)
                v.tensor_tensor(t2[:, a:b, :], src[:, a:b, :],
                                tdma[:, a:b, :], op=ALU.add)
                shift_h(tdma, src, a, b, up=True)
                if two_center:
                    v.tensor_tensor(dst[:, a:b, :], src[:, a:b, :],
                                    tdma[:, a:b, :], op=ALU.add)
                    v.tensor_tensor(dst[:, a:b, :][:, :, :],
                                    t2[:, a:b, :], dst[:, a:b, :],
                                    op=ALU.add) if False else None
                    v.tensor_tensor(tdma[:, a:b, :], t2[:, a:b, :],
                                    dst[:, a:b, :], op=ALU.add)
                    sc.copy(dst[:, a:b, :], tdma[:, a:b, :])
                else:
                    v.tensor_tensor(dst[:, a:b, :], t2[:, a:b, :],
                                    tdma[:, a:b, :], op=ALU.add)

            def grads(src, ox, oy, oz):
                d1_w(sA, src, 0, EZ)
                s1_d(sB, sA, sC, c0, c1)
                s_h(ox, sB, dmt, sC, c0, c1, two_center=True)
                s2_d(sB, sA, sC, c0, c1)
                s_h(oy, sB, dmt, sC, c0, c1, two_center=False)
                s_h(sB, src, dmt, sC, 0, EZ, two_center=False)
                s2_w(sA, sB, sC, 0, EZ)
                v.tensor_tensor(oz[:, c0:c1, :], sA[:, c0 + 1:c1 + 1, :],
                                sA[:, c0 - 1:c1 - 1, :], op=ALU.subtract)

            grads(x0zt, gx, gy, gz)
            grads(yzt, tx, ty, tz)
            # np2 -> sA, nt2 -> sB, dot -> sC (core planes only)
            cc = (slice(None), slice(c0, c1), slice(None))
            sc.square(dmt[cc], gx[cc])
            sc.square(x0zt[cc], gy[cc])
            v.tensor_tensor(sA[cc], dmt[cc], x0zt[cc], op=ALU.add)
            sc.square(dmt[cc], gz[cc])
            v.tensor_tensor(sA[cc], sA[cc][:, :, :], dmt[cc], op=ALU.add) \
                if False else v.tensor_tensor(x0zt[cc], sA[cc], dmt[cc],
                                              op=ALU.add)
            sA, x0zt = x0zt, sA  # np2 now in (renamed) sA
            sc.square(dmt[cc], tx[cc])
            sc.square(yzt[cc], ty[cc])
            v.tensor_tensor(sB[cc], dmt[cc], yzt[cc], op=ALU.add)
            sc.square(dmt[cc], tz[cc])
            v.tensor_tensor(yzt[cc], sB[cc], dmt[cc], op=ALU.add)
            sB, yzt = yzt, sB  # nt2 now in sB
            v.tensor_tensor(dmt[cc], gx[cc], tx[cc], op=ALU.mult)
            v.tensor_tensor(x0zt[cc], gy[cc], ty[cc], op=ALU.mult)
            v.tensor_tensor(sC[cc], dmt[cc], x0zt[cc], op=ALU.add)
            v.tensor_tensor(dmt[cc], gz[cc], tz[cc], op=ALU.mult)
            v.tensor_tensor(x0zt[cc], sC[cc], dmt[cc], op=ALU.add)
            sC, x0zt = x0zt, sC  # dot now in sC
            sc.sqrt(gx[cc], sA[cc])   # npn
            sc.sqrt(tx[cc], sB[cc])   # ntn
            v.tensor_scalar(gy[cc], gx[cc], 1e-12, None, op0=ALU.max)
            v.reciprocal(gz[cc], gy[cc])          # inv_p
            v.tensor_scalar(ty[cc], tx[cc], 1e-12, None, op0=ALU.max)
            v.reciprocal(tz[cc], ty[cc])          # inv_t
            v.tensor_tensor(dmt[cc], sC[cc], gz[cc], op=ALU.mult)
            v.tensor_tensor(sC[cc], dmt[cc], tz[cc], op=ALU.mult)   # num
            v.tensor_tensor(gy[cc], gx[cc], gz[cc], op=ALU.mult)    # npn*invp
            v.tensor_tensor(ty[cc], tx[cc], tz[cc], op=ALU.mult)    # ntn*invt
            v.tensor_tensor(dmt[cc], gy[cc], ty[cc], op=ALU.mult)
            v.tensor_scalar(gy[cc], dmt[cc], 1e-8, None, op0=ALU.max)  # den
            v.reciprocal(gz[cc], gy[cc])
            v.scalar_tensor_tensor(dmt[cc], sC[cc], 1.0, gz[cc],
                                   op0=ALU.mult, op1=ALU.mult,
                                   accum_out=col(S_DIR))

        # ------------- finalize --------------------------------------------
        prs = pp.tile([128, NS], F32, tag="prs")
        gp.partition_all_reduce(prs[:], cols[:], channels=128,
                                reduce_op=bass_isa.ReduceOp.add)
        nc.sync.dma_start(out=sums_out[:], in_=prs[0:1, :])


# ------------------------------ host side ----------------------------------

def _rep_slab(vol, lo, hi):
    idx = np.clip(np.arange(lo, hi), 0, vol.shape[0] - 1)
    return np.ascontiguousarray(vol[idx].transpose(1, 0, 2)).reshape(128, -1)


def _band128():
    # symmetric 3-tap H-sum band; edge rows double-count (replicate pad)
    b = np.zeros((128, 128), np.float32)
    i = np.arange(128)
    b[i, i] = 1.0
    b[i[:-1], i[:-1] + 1] = 1.0
    b[i[1:], i[1:] - 1] = 1.0
    b[0, 0] = 2.0
    b[127, 127] = 2.0
    return b


def _zero_slab(vol, lo, hi):
    out = np.zeros((hi - lo, H, W), np.float32)
    a, b = max(lo, 0), min(hi, D)
    out[a - lo:b - lo] = vol[a:b]
    return np.ascontiguousarray(out.transpose(1, 0, 2)).reshape(128, -1)


def _in_maps(net_output, target):
    maps = []
    for c in range(N_CORES):
        b, q = c // 4, c % 4
        c0 = 16 * q
        lo, hi = c0 - HALO, c0 + CW + HALO
        x0 = np.asarray(net_output[b, 0], np.float32)
        x1 = np.asarray(net_output[b, 1], np.float32)
        tg = (np.asarray(target[b, 0]) > 0).astype(np.float32)
        sel = np.zeros((1, 8), np.float32)
        neg = np.full((1, 8), -3.0e38, np.float32)
        s01 = np.zeros((1, 8), np.float32)
        # AR slot layout: quantity i (maxT,maxP,negminT,negminP) of batch b
        # lives at slot 4*b+i; arin is my4 tiled twice so tiled[4b+i]=my4[i].
        for i in range(4):
            sel[0, 4 * b + i] = 1.0
            neg[0, 4 * b + i] = 0.0
            s01[0, 4 * b + i] = 1.0
        maps.append({
            "x0e": _rep_slab(x0, lo, hi),
            "x1e": _rep_slab(x1, lo, hi),
            "tge": _rep_slab(tg, lo, hi),
            "x0z": _zero_slab(x0, c0 - 1, c0 + CW + 1),
            "yz": _zero_slab(tg, c0 - 1, c0 + CW + 1),
            "selv": sel, "negv": neg, "sel01": s01,
            "band": _band128(), "ident": np.eye(128, dtype=np.float32),
        })
    return maps


def _combine(parts):
    T = np.sum(np.stack(parts, 0), axis=0)[0].astype(np.float64)
    N = float(B * D * H * W)
    dice = -((2 * T[S_PROBY] + 1e-5) / (T[S_PROB] + T[S_Y] + 1e-5))
    ce = (T[S_SOFTPLUS] - T[S_YD]) / N
    tprec = (T[S_SKPY] + 1.0) / (T[S_SKP] + 1.0)
    tsens = (T[S_SKTP] + 1.0) / (T[S_SKT] + 1.0)
    cl = 1.0 - 2.0 * tprec * tsens / (tprec + tsens)
    dirl = 1.0 - T[S_DIR] / N
    conn = (T[S_CONN0] + T[S_CONN1]) / (2 * N)
    g1 = 1.0 - (T[S_INTER1] + 1.0) / (0.1 * T[S_QSP2] + 0.9 * T[S_QSPQVL] + 1.0)
    g2 = 1.0 - (T[S_INTER2] + 1.0) / (0.1 * T[S_QSLQVP] + 0.9 * T[S_QSL2] + 1.0)
    return np.float32(dice + ce + cl + dirl + conn + g1 + g2)


def kernel(net_output, target, t_skeletonize_flage=None):
    global _CACHED_NC
    if _CACHED_NC is None:
        _CACHED_NC = _build_nc()
    nc = _CACHED_NC
    maps = _in_maps(np.asarray(net_output), np.asarray(target))
    trace = bool(int(os.environ.get("KERNEL_TRACE", "0")))
    res = run_bass_kernel_spmd(nc, maps, core_ids=list(range(N_CORES)),
                               trace=trace)
    if trace and res.exec_time_ns is not None:
        print(f"HW exec time: {res.exec_time_ns} ns")
        kernel.last_exec_ns = res.exec_time_ns
    parts = [res.results[c]["sums"] for c in range(N_CORES)]
    kernel.last_parts = parts
    return _combine(parts)

